# revision 1
# baseline (speedup 1.0000x reference)
"""RWKV WKV attention block on 8 Trainium2 NeuronCores.

Sharding: data-parallel over batch B=8 -> one batch element per core.
Per-core pipeline (T=2048 in chunks of Tc=256):
  x^T (pre-transposed on host) -> time-mix (ACT+DVE) -> K/V/R projections
  (PE, bf16 with fp32 PSUM accumulate; RWKV_F32R=1 env selects fp32r) ->
  exp/sigmoid (ACT, read PSUM directly, fp32) -> WKV linear recurrence
  (fp32 DVE tensor_tensor_scan along T, elementwise tail split across
  DVE/Pool) -> output projection (PE) -> natural-layout DMA out.
  Deep tile-pool buffering keeps the per-(chunk, channel-group) pipeline
  chains overlapped across engines.
All weights are pre-transposed on the host so no on-device transposes are
needed (contraction dim lands on partitions for every matmul).
"""

import sys

for _p in ("/opt/trn_rl_repo", "/root/.axon_site/_ro/trn_rl_repo"):
    if _p not in sys.path:
        sys.path.append(_p)

import numpy as np

import concourse.bass as bass
import concourse.mybir as mybir
import concourse.tile as tile
from concourse.bass_utils import run_bass_kernel_spmd

F32 = mybir.dt.float32
F32R = mybir.dt.float32r
import os
MMDT = mybir.dt.float32r if os.environ.get("RWKV_F32R") else mybir.dt.bfloat16
ALU = mybir.AluOpType
ACT_F = mybir.ActivationFunctionType

B, T, D = 8, 2048, 1024
P = 128
G = D // P          # 8 channel groups
TC = 256            # T chunk
NCH = T // TC       # 8 chunks
TS = TC // P        # t-subtiles per chunk in the output projection


def _split_waits(nc, maxw=1):
    """walrus in this image rejects >1 sync-wait per instruction; move the
    excess onto preceding same-engine no-ops (semantically identical)."""
    for f in nc.m.functions:
        for bb in f.blocks:
            new_insts = []
            for ins in bb.instructions:
                si = ins.sync_info
                if si is not None and si.on_wait and len(si.on_wait) > maxw:
                    waits = list(si.on_wait)
                    extra, keep = waits[:-maxw], waits[-maxw:]
                    for i in range(0, len(extra), maxw):
                        nop = mybir.InstNoOp(name=f"{ins.name}-ws{i}", ins=[], outs=[])
                        nop.engine = ins.engine
                        nop.sync_info = mybir.SyncInfo(
                            on_wait=extra[i:i + maxw], on_update=[])
                        new_insts.append(nop)
                        nc.register_instruction(nop, overwrite=True)
                    si.on_wait = keep
                new_insts.append(ins)
            bb.instructions = new_insts


def _build_nc(reps=None):
    nc = bass.Bass()

    xT = nc.declare_dram_parameter("xT", [P, G, T + 1], F32, isOutput=False)
    WkT = nc.declare_dram_parameter("WkT", [P, G, D], MMDT, isOutput=False)
    WvT = nc.declare_dram_parameter("WvT", [P, G, D], MMDT, isOutput=False)
    WrT = nc.declare_dram_parameter("WrT", [P, G, D], MMDT, isOutput=False)
    WoT = nc.declare_dram_parameter("WoT", [P, G, D], MMDT, isOutput=False)
    ew_p = nc.declare_dram_parameter("ew", [P, G], F32, isOutput=False)
    etf_p = nc.declare_dram_parameter("etf", [P, G], F32, isOutput=False)
    tmk_p = nc.declare_dram_parameter("tmk", [P, G], F32, isOutput=False)
    tmv_p = nc.declare_dram_parameter("tmv", [P, G], F32, isOutput=False)
    tmr_p = nc.declare_dram_parameter("tmr", [P, G], F32, isOutput=False)
    tmk1_p = nc.declare_dram_parameter("tmk1", [P, G], F32, isOutput=False)
    tmv1_p = nc.declare_dram_parameter("tmv1", [P, G], F32, isOutput=False)
    tmr1_p = nc.declare_dram_parameter("tmr1", [P, G], F32, isOutput=False)
    ln_p = nc.declare_dram_parameter("lnum", [P, G], F32, isOutput=False)
    ld_p = nc.declare_dram_parameter("lden", [P, G], F32, isOutput=False)
    out_p = nc.declare_dram_parameter("out", [T, D], F32, isOutput=True)

    with tile.TileContext(nc) as tc:
        with tc.tile_pool(name="wts", bufs=1) as wts, \
             tc.tile_pool(name="consts", bufs=1) as consts, \
             tc.tile_pool(name="xs", bufs=2) as xs, \
             tc.tile_pool(name="mix", bufs=2) as mixp, \
             tc.tile_pool(name="scan", bufs=1) as scanp, \
             tc.tile_pool(name="tr", bufs=4) as tr, \
             tc.tile_pool(name="tr1", bufs=3) as tr1, \
             tc.tile_pool(name="wsrp", bufs=2) as wsrp, \
             tc.tile_pool(name="wop", bufs=1) as wop, \
             tc.tile_pool(name="outp", bufs=2) as outp, \
             tc.tile_pool(name="pskvr", bufs=2, space="PSUM") as pskvr, \
             tc.tile_pool(name="psout", bufs=2, space="PSUM") as psout:

            def emit_all():
                # DMA emission order matters for startup latency: chunk-0 x and
                # the small consts first, then Wk (first weights the PE needs),
                # then Wv/Wr, then Wo (only needed at end of chunk 0).
                def load_xw(c):
                    t = xs.tile([P, G, TC + 1], F32, tag="xw")
                    nc.sync.dma_start(t[:], xT[:, :, c * TC:c * TC + TC + 1])
                    return t

                xw_next = load_xw(0)

                ew_sb = consts.tile([P, G], F32, tag="ew")
                etf_sb = consts.tile([P, G], F32, tag="etf")
                tmk_sb = consts.tile([P, G], F32, tag="tmk")
                tmv_sb = consts.tile([P, G], F32, tag="tmv")
                tmr_sb = consts.tile([P, G], F32, tag="tmr")
                tmk1_sb = consts.tile([P, G], F32, tag="tmk1")
                tmv1_sb = consts.tile([P, G], F32, tag="tmv1")
                tmr1_sb = consts.tile([P, G], F32, tag="tmr1")
                ln_sb = consts.tile([P, G], F32, tag="ln")
                ld_sb = consts.tile([P, G], F32, tag="ld")
                nc.sync.dma_start(tmk_sb[:], tmk_p[:])
                nc.sync.dma_start(tmv_sb[:], tmv_p[:])
                nc.sync.dma_start(tmr_sb[:], tmr_p[:])
                nc.sync.dma_start(tmk1_sb[:], tmk1_p[:])
                nc.sync.dma_start(tmv1_sb[:], tmv1_p[:])
                nc.sync.dma_start(tmr1_sb[:], tmr1_p[:])
                nc.sync.dma_start(ew_sb[:], ew_p[:])
                nc.sync.dma_start(etf_sb[:], etf_p[:])
                nc.sync.dma_start(ln_sb[:], ln_p[:])
                nc.sync.dma_start(ld_sb[:], ld_p[:])

                wk = wts.tile([P, G, D], MMDT, tag="wk")
                wv = wts.tile([P, G, D], MMDT, tag="wv")
                wr = wts.tile([P, G, D], MMDT, tag="wr")
                for ig in range(G):
                    nc.sync.dma_start(wk[:, ig], WkT[:, ig])
                for ig in range(G):
                    nc.sync.dma_start(wv[:, ig], WvT[:, ig])
                for ig in range(G):
                    nc.sync.dma_start(wr[:, ig], WrT[:, ig])

                # persistent scan state buffers: [p, jg, 1+TC]; col 0 = carry-in
                numb = scanp.tile([P, G, 1 + TC], F32, tag="numb")
                denb = scanp.tile([P, G, 1 + TC], F32, tag="denb")

                # Wo resident (loaded last; first needed at end of chunk 0)
                wo = wop.tile([P, G, D], MMDT, tag="wo")
                for ig in range(G):
                    nc.sync.dma_start(wo[:, ig], WoT[:, ig])

                def mix3(xw, ig, which, out_tiles):
                    """xm = tm*cur + (1-tm)*sh: ACT does tm*cur (per-partition
                    scale), DVE folds in (1-tm)*sh with one STT."""
                    cur = xw[:, ig, 1:1 + TC]
                    sh = xw[:, ig, 0:TC]
                    for nm, tm, tm1 in which:
                        t1 = tr.tile([P, TC], F32, tag=f"m{nm}")
                        nc.scalar.mul(t1[:], cur, tm[:, ig:ig + 1])
                        t = mixp.tile([P, TC], MMDT, tag=f"{nm}{ig}")
                        nc.vector.scalar_tensor_tensor(
                            t[:], sh, tm1[:, ig:ig + 1], t1[:], ALU.mult, ALU.add)
                        out_tiles[nm].append(t)

                for c in range(NCH):
                    t0 = c * TC

                    # x^T window [P, G, TC+1]: col 0 is t0-1 (or last_x for c=0)
                    xw = xw_next
                    if c + 1 < NCH:
                        xw_next = load_xw(c + 1)

                    # time-mix on Pool/DVE (alternating), per-ig tiles for
                    # fine-grained deps; pass-split so each mix tensor's last PE
                    # use is early, letting next chunk's mixes overlap this one.
                    mixes = {"xk": [], "xv": [], "xr": []}
                    for ig in range(G):
                        mix3(xw, ig, [("xk", tmk_sb, tmk1_sb),
                                      ("xv", tmv_sb, tmv1_sb),
                                      ("xr", tmr_sb, tmr1_sb)], mixes)
                    xk, xv, xr = mixes["xk"], mixes["xv"], mixes["xr"]

                    # carry-in columns for all jg at once (strided copy)
                    if c == 0:
                        nc.vector.tensor_copy(numb[:, :, 0], ln_sb[:, :])
                        nc.vector.tensor_copy(denb[:, :, 0], ld_sb[:, :])
                    else:
                        nc.vector.tensor_copy(numb[:, :, 0], numb[:, :, TC])
                        nc.vector.tensor_copy(denb[:, :, 0], denb[:, :, TC])

                    # ---- pass KV: k & v projections + WKV scan -> wsr=wkv ----
                    wsr = wsrp.tile([P, G, TC], MMDT, tag="wsr")
                    for jg in range(G):
                        jsl = bass.ts(jg, P)
                        kps = pskvr.tile([P, TC], F32, tag="kps")
                        for ig in range(G):
                            nc.tensor.matmul(kps[:], wk[:, ig, jsl], xk[ig][:],
                                             start=(ig == 0), stop=(ig == G - 1))
                        vps = pskvr.tile([P, TC], F32, tag="vps")
                        for ig in range(G):
                            nc.tensor.matmul(vps[:], wv[:, ig, jsl], xv[ig][:],
                                             start=(ig == 0), stop=(ig == G - 1))
                        ekt = tr.tile([P, TC], F32, tag="ek")
                        ek = ekt[:]
                        nc.scalar.activation(ek, kps[:], ACT_F.Exp)
                        ekv = tr1.tile([P, TC], F32, tag="ekv")
                        nc.vector.tensor_mul(ekv[:], ek, vps[:])

                        ewb = ew_sb[:, jg:jg + 1].to_broadcast([P, TC])
                        nc.vector.tensor_tensor_scan(
                            numb[:, jg, 1:1 + TC], ewb, ekv[:],
                            numb[:, jg, 0:1], ALU.mult, ALU.add)
                        nc.vector.tensor_tensor_scan(
                            denb[:, jg, 1:1 + TC], ewb, ek,
                            denb[:, jg, 0:1], ALU.mult, ALU.add)

                        # euk = ek*etf, so:
                        # numer = num_{t-1} + etf*ekv ; denom = den_{t-1} + etf*ek
                        numer = tr1.tile([P, TC], F32, tag="numer")
                        denom = tr1.tile([P, TC], F32, tag="denom")
                        etfs = etf_sb[:, jg:jg + 1]
                        nc.vector.scalar_tensor_tensor(
                            numer[:], ekv[:], etfs, numb[:, jg, 0:TC],
                            ALU.mult, ALU.add)
                        nc.vector.scalar_tensor_tensor(
                            denom[:], ek, etfs, denb[:, jg, 0:TC],
                            ALU.mult, ALU.add)
                        nc.vector.reciprocal(denom[:], denom[:])
                        nc.gpsimd.tensor_mul(wsr[:, jg], numer[:], denom[:])

                    # ---- pass R: r projections, wsr *= sigmoid(r) ----
                    for jg in range(G):
                        jsl = bass.ts(jg, P)
                        rps = pskvr.tile([P, TC], F32, tag="rps")
                        for ig in range(G):
                            nc.tensor.matmul(rps[:], wr[:, ig, jsl], xr[ig][:],
                                             start=(ig == 0), stop=(ig == G - 1))
                        sr = tr1.tile([P, TC], F32, tag="sr")
                        nc.scalar.activation(sr[:], rps[:], ACT_F.Sigmoid)
                        if MMDT == F32R:
                            nc.gpsimd.tensor_mul(wsr[:, jg], wsr[:, jg].bitcast(F32), sr[:])
                        else:
                            nc.gpsimd.tensor_mul(wsr[:, jg], wsr[:, jg], sr[:])

                    # ---- pass O: out[t, d] = sum_j wsr[j, t] * WoT[j, d] ----
                    for dt in range(2):
                        for ts in range(TS):
                            ops = psout.tile([P, 512], F32, tag="ops")
                            for jg in range(G):
                                nc.tensor.matmul(
                                    ops[:], wsr[:, jg, bass.ts(ts, P)],
                                    wo[:, jg, bass.ts(dt, 512)],
                                    start=(jg == 0), stop=(jg == G - 1))
                            ob = outp.tile([P, 512], F32, tag="ob")
                            nc.scalar.copy(ob[:], ops[:])
                            nc.sync.dma_start(
                                out_p[t0 + ts * P:t0 + (ts + 1) * P,
                                      bass.ts(dt, 512)], ob[:])


            for _ in range(reps or 1):
                emit_all()

    _split_waits(nc, 1)
    return nc


_NC_CACHE = None


def _get_nc():
    global _NC_CACHE
    if _NC_CACHE is None:
        _NC_CACHE = _build_nc()
    return _NC_CACHE


def _pg(v):
    """(D,) channel vector -> [P, G] with channel d = g*128 + p."""
    return np.ascontiguousarray(np.asarray(v, np.float32).reshape(G, P).T)


def _wt(w):
    """W (D_out, D_in) -> W.T tiled [P, G, D_out] (contraction on partitions)."""
    wt = np.asarray(w, np.float32).T  # (D_in, D_out)
    out = np.ascontiguousarray(wt.reshape(G, P, D).transpose(1, 0, 2))
    return out.astype(mybir.dt.np(MMDT))


def kernel(x, last_x, last_num, last_den, time_decay, time_first,
           time_mix_k, time_mix_v, time_mix_r, Wk, Wv, Wr, Wo):
    x = np.asarray(x, np.float32)
    last_x = np.asarray(last_x, np.float32)
    last_num = np.asarray(last_num, np.float32)
    last_den = np.asarray(last_den, np.float32)

    ew = _pg(np.exp(-np.exp(np.asarray(time_decay, np.float64))))
    etf = _pg(np.exp(np.asarray(time_first, np.float64)))
    tmk = _pg(np.asarray(time_mix_k).reshape(-1))
    tmv = _pg(np.asarray(time_mix_v).reshape(-1))
    tmr = _pg(np.asarray(time_mix_r).reshape(-1))
    wkT, wvT, wrT, woT = _wt(Wk), _wt(Wv), _wt(Wr), _wt(Wo)

    in_maps = []
    for b in range(B):
        xs = np.concatenate([last_x[b], x[b]], axis=0)      # (T+1, D)
        xTb = np.ascontiguousarray(
            xs.T.reshape(G, P, T + 1).transpose(1, 0, 2))   # [P, G, T+1]
        in_maps.append({
            "xT": xTb,
            "WkT": wkT, "WvT": wvT, "WrT": wrT, "WoT": woT,
            "ew": ew, "etf": etf, "tmk": tmk, "tmv": tmv, "tmr": tmr,
            "tmk1": 1.0 - tmk, "tmv1": 1.0 - tmv, "tmr1": 1.0 - tmr,
            "lnum": _pg(last_num[b, 0]), "lden": _pg(last_den[b, 0]),
        })

    global _last_in_maps
    _last_in_maps = in_maps
    nc = _get_nc()
    res = run_bass_kernel_spmd(nc, in_maps, list(range(B)))
    return np.stack([res.results[b]["out"] for b in range(B)], axis=0)


_last_in_maps = None

